# revision 1
# baseline (speedup 1.0000x reference)
"""Trainium2 Bass kernel for nn_CVNonGaussianQuantumLayer.

12-qubit batched state-vector simulator, batch 128, two circuits
(X-measured and Z-measured). Data-parallel over 8 cores: 16 batch rows
per core; each core simulates its rows for BOTH circuits (32 states).

Algorithm (all index math verified against the jax reference on host):
  - wires 0..6 (hi, 7 bits -> h = j>>5) live on SBUF partitions (layout A),
    wires 7..11 (lo, 5 bits -> l = j&31) + state id s on the free dim.
  - All hi-wire gates of a layer (fused 1q U's, CRX(0..5), chain CNOTs
    (0,1)..(4,5)) compose on host into one 128x128 complex matrix H_L,
    applied with PE matmuls in layout A.
  - PE 128x128-block transposes move the state to layout B
    (partitions = (s mod 4, lo), free = (s//4, h)).
  - CRX(6,7) = round-1 matmuls on the b6(h)=1 column set only.
  - CNOT(5,6) = pi56 column permutation folded into the psum->sbuf copies.
  - lo-wire gates + CNOT(6,7) + lo chain CNOTs compose into M2(b6) 32x32
    matrices (expanded I4 (x) M2), applied as round-2 column-group matmuls.
  - Final layer: circuit 0 folds H(x)12 (Hadamard trick: <X_w> = <Z_w> of
    H psi); H_lo into last M2, H_hi as a final matmul after transposing
    back. Both measurements become diagonal: |amp|^2 partial sums are
    DMA'd out and the signed reductions happen on host (tiny).
"""
import sys
import numpy as np

if '/opt/trn_rl_repo' not in sys.path:
    sys.path.insert(0, '/opt/trn_rl_repo')

NQ, NL = 12, 2
NCORES, BPC = 8, 16          # cores, batch rows per core
NHI, NLO = 7, 5
DHI, DLO = 128, 32

F32 = None  # set lazily (mybir import)


# ---------------- host math ----------------
def _rx(th):
    h = 0.5 * th
    return np.array([[np.cos(h), -1j * np.sin(h)], [-1j * np.sin(h), np.cos(h)]])


def _ry(th):
    h = 0.5 * th
    return np.array([[np.cos(h), -np.sin(h)], [np.sin(h), np.cos(h)]])


def _rz(th):
    e = np.exp(-0.5j * th)
    return np.array([[e, 0], [0, np.conj(e)]])


def _phase(phi):
    return np.array([[1, 0], [0, np.exp(1j * phi)]])


def _sigmoid(v):
    return 1.0 / (1.0 + np.exp(-v))


def _fused_u(r3, t1):
    return _phase(_sigmoid(t1) * np.pi) @ _rz(r3[2]) @ _ry(r3[1]) @ _rx(r3[0])


def _kron_at(U, w, n):
    M = np.eye(1, dtype=complex)
    for k in range(n):
        M = np.kron(M, U if k == w else np.eye(2))
    return M


def _kron2_at(U4, w, n):
    M = np.eye(1, dtype=complex)
    k = 0
    while k < n:
        if k == w:
            M = np.kron(M, U4)
            k += 2
        else:
            M = np.kron(M, np.eye(2))
            k += 1
    return M


def _crx4(th):
    U = np.eye(4, dtype=complex)
    U[2:, 2:] = _rx(th)
    return U


_CNOT4 = np.array([[1, 0, 0, 0], [0, 1, 0, 0], [0, 0, 0, 1], [0, 0, 1, 0]],
                  dtype=complex)


def _build_mats_one_circuit(rot, cx, t):
    """Per layer: H (128x128), RX7 theta, M2_0, M2_1 (32x32)."""
    out = []
    for L in range(NL):
        H = np.eye(DHI, dtype=complex)
        for w in range(NHI):
            H = _kron_at(_fused_u(rot[L, w], t[L, w]), w, NHI) @ H
            if w <= 5:
                H = _kron2_at(_crx4(_sigmoid(cx[L, w]) * np.pi), w, NHI) @ H
        for w in range(5):
            H = _kron2_at(_CNOT4, w, NHI) @ H
        th67 = _sigmoid(cx[L, 6]) * np.pi
        R = np.eye(DLO, dtype=complex)
        for w in range(NHI, NQ):
            R = _kron_at(_fused_u(rot[L, w], t[L, w]), w - NHI, NLO) @ R
            if w <= NQ - 2:
                R = _kron2_at(_crx4(_sigmoid(cx[L, w]) * np.pi), w - NHI, NLO) @ R
        chainlo = np.eye(DLO, dtype=complex)
        for w in range(4):
            chainlo = _kron2_at(_CNOT4, w, NLO) @ chainlo
        X7 = _kron_at(np.array([[0, 1], [1, 0]], dtype=complex), 0, NLO)
        out.append(dict(H=H, th67=th67, M2_0=chainlo @ R, M2_1=chainlo @ X7 @ R))
    return out


def _hadamards():
    Hd = np.array([[1, 1], [1, -1]], dtype=complex) / np.sqrt(2)
    Hhi = np.eye(1, dtype=complex)
    Hlo = np.eye(1, dtype=complex)
    for _ in range(NHI):
        Hhi = np.kron(Hhi, Hd)
    for _ in range(NLO):
        Hlo = np.kron(Hlo, Hd)
    return Hhi, Hlo


def _initial_state(x):
    ang = np.arctan2(x, 1.0) * np.pi
    h = 0.5 * ang
    c, s = np.cos(h), np.sin(h)
    psi = np.ones((x.shape[0], 1))
    for w in range(NQ):
        vec = np.stack([c[:, w], s[:, w]], axis=-1)
        psi = (psi[:, :, None] * vec[:, None, :]).reshape(x.shape[0], -1)
    return psi  # real [B, 4096]


def _weight_pack(rotations, cx_strengths, t_gates):
    """Build the [NW, 128, 128] f32 weight tensor + index map."""
    matsx = _build_mats_one_circuit(rotations[0], cx_strengths[0], t_gates[0])
    matsz = _build_mats_one_circuit(rotations[1], cx_strengths[1], t_gates[1])
    Hhi, Hlo = _hadamards()
    # fold H_lo into circuit-0 last-layer M2's
    matsx[NL - 1]['M2_0'] = Hlo @ matsx[NL - 1]['M2_0']
    matsx[NL - 1]['M2_1'] = Hlo @ matsx[NL - 1]['M2_1']

    mats = []
    idx = {}
    cosvals = {}

    def add(key, M):
        idx[key] = len(mats)
        mats.append(np.ascontiguousarray(M, dtype=np.float32))

    I16 = np.eye(16)
    I4 = np.eye(4)
    X2 = np.array([[0., 1.], [1., 0.]])
    for L in range(NL):
        for c, mm in ((0, matsx), (1, matsz)):
            m = mm[L]
            H = m['H']
            add(('A_rT', L, c), H.real.T)
            add(('A_iT', L, c), H.imag.T)
            if L > 0:
                add(('A_negiT', L, c), -H.imag.T)
            # round-1 RX on bit7: full = I4 (x) kron(rx(th), I16)
            th = m['th67']
            cosI = np.eye(128) * np.cos(0.5 * th)
            sinX = np.kron(I4, np.kron(X2, I16)) * np.sin(0.5 * th)
            add(('R1_cos', L, c), cosI)      # diagonal
            add(('R1_sinX', L, c), sinX)     # symmetric
            add(('R1_negsinX', L, c), -sinX)
            for b6 in (0, 1):
                M2 = np.kron(I4, m[f'M2_{b6}'])
                add(('M2_rT', L, c, b6), M2.real.T)
                add(('M2_iT', L, c, b6), M2.imag.T)
                add(('M2_negiT', L, c, b6), -M2.imag.T)
    add(('Hhi',), Hhi.real)                  # symmetric real
    add(('ident',), np.eye(128))
    misc = np.zeros((128, 128), dtype=np.float32)
    for (L, c), v in cosvals.items():
        misc[:, 2 * L + c] = v
    add(('misc',), misc)
    return np.stack(mats), idx


# ---------------- device program ----------------
_CACHE = {}


def _build_program(nw):
    import concourse.bass as bass
    import concourse.mybir as mybir
    import concourse.tile as tile

    from concourse.tile_rust import add_dep_helper
    F32 = mybir.dt.float32
    BF16 = mybir.dt.bfloat16
    MULT = mybir.AluOpType.mult
    ADD = mybir.AluOpType.add
    SQ = mybir.ActivationFunctionType.Square
    nc = bass.Bass()
    st0_ext = nc.declare_dram_parameter("st0", [128, 1024], F32, isOutput=False)
    wts_ext = nc.declare_dram_parameter("wts", [nw, 128, 128], F32, isOutput=False)
    p2x_ext = nc.declare_dram_parameter("p2x", [128, 512], F32, isOutput=True)
    p2z_ext = nc.declare_dram_parameter("p2z", [128, 512], F32, isOutput=True)

    widx = _build_program.widx  # set by caller

    with tile.TileContext(nc) as tc:
        with (
            tc.tile_pool(name="lpool", bufs=1) as lpool,
            tc.tile_pool(name="wpool", bufs=1) as wpool,
            tc.tile_pool(name="spool", bufs=2) as spool,
            tc.tile_pool(name="opool", bufs=1) as opool,
            tc.tile_pool(name="ppool", bufs=6, space="PSUM") as ppool,
            tc.tile_pool(name="tpool", bufs=2, space="PSUM") as tpool,
        ):
            # Weights: DMA to a landing tile, then DVE-copy to the tile PE
            # reads. Every producer PE sees is then the DVE engine, so each
            # matmul needs at most ONE semaphore wait (fp32 matmuls encode
            # only one) -- the fused-LdWeights HW slot limit.
            W = {}

            def getw(key):
                if key not in W:
                    i = widx[key]
                    land = lpool.tile([128, 128], F32, tag=f"land{i}")
                    dma(nc.sync, out=land[:], in_=wts_ext[i])
                    t = wpool.tile([128, 128], F32, tag=f"w{i}")
                    nc.vector.tensor_copy(t[:], land[:])
                    W[key] = t
                return W[key]

            last_copy = [None]
            dma_insts = []

            def dma(eng, **kw):
                dma_insts.append(eng.dma_start(**kw))
                return dma_insts[-1]

            def copy(out, in_):
                last_copy[0] = nc.vector.tensor_copy(out, in_)
                return last_copy[0]

            # initial state (real): DMA -> landing -> DVE copy
            st_land = lpool.tile([128, 1024], F32, tag="sland")
            dma(nc.sync, out=st_land[:], in_=st0_ext[:])
            stA_r = spool.tile([128, 1024], F32, tag="stAr")
            copy(stA_r[:], st_land[:])
            stA_i = None
            ident = getw(('ident',))
            misc = getw(('misc',))

            jw = wpool.tile([128, 8], BF16, tag="jw")
            nc.vector.memset(jw[:], 0)

            def absorb(ap=None):
                # PE clock absorber: bf16 ldweights writes no psum (no WAW)
                # and we force exactly one dependency -- the newest DVE
                # copy -- so it always encodes a single semaphore wait,
                # pre-acquiring the DVE tick for the following matmuls.
                ld = nc.tensor.ldweights(jw[:])
                if last_copy[0] is not None:
                    add_dep_helper(ld.ins, last_copy[0].ins,
                                   reason="absorb newest DVE tick")

            def cmm(ps, lhsT_list, rhs_list):
                n = len(lhsT_list)
                for k, (lt, rh) in enumerate(zip(lhsT_list, rhs_list)):
                    nc.tensor.matmul(ps, lt[:], rh, start=(k == 0),
                                     stop=(k == n - 1))

            for L in range(NL):
                # ===== A phase =====
                if L:
                    absorb(stA_i[:, 896:])
                stApost_r = spool.tile([128, 1024], F32, tag="sApr")
                stApost_i = spool.tile([128, 1024], F32, tag="sApi")
                for c in range(2):
                    cols = slice(512 * c, 512 * (c + 1))
                    ps_r = ppool.tile([128, 512], F32, tag="ps")
                    ps_i = ppool.tile([128, 512], F32, tag="ps")
                    if L == 0:
                        cmm(ps_r[:], [getw(('A_rT', L, c))], [stA_r[:, cols]])
                        cmm(ps_i[:], [getw(('A_iT', L, c))], [stA_r[:, cols]])
                    else:
                        cmm(ps_r[:], [getw(('A_rT', L, c)),
                                      getw(('A_negiT', L, c))],
                            [stA_r[:, cols], stA_i[:, cols]])
                        cmm(ps_i[:], [getw(('A_iT', L, c)),
                                      getw(('A_rT', L, c))],
                            [stA_r[:, cols], stA_i[:, cols]])
                    copy(stApost_r[:, cols], ps_r[:])
                    copy(stApost_i[:, cols], ps_i[:])

                # ===== transpose A -> B =====
                B0_r = spool.tile([128, 1024], F32, tag="B0r")
                B0_i = spool.tile([128, 1024], F32, tag="B0i")
                absorb(stApost_i[:, 896:])
                for m in range(8):
                    cs = slice(128 * m, 128 * (m + 1))
                    for srct, dst in ((stApost_r, B0_r), (stApost_i, B0_i)):
                        pt = tpool.tile([128, 128], F32, tag="pt")
                        nc.tensor.transpose(pt[:], srct[:, cs], ident[:])
                        copy(dst[:, cs], pt[:])

                B0v_r = B0_r[:].rearrange("p (m h q) -> p m h q", m=8, h=32, q=4)
                B0v_i = B0_i[:].rearrange("p (m h q) -> p m h q", m=8, h=32, q=4)

                # ===== round 1: RX7 on b6=1 columns =====
                absorb(B0_i[:, 896:])
                ps1 = {}
                for c in range(2):
                    mc = slice(4 * c, 4 * (c + 1))
                    xr = B0v_r[:, mc, :, 1::2]   # [128,4,32,2]
                    xi = B0v_i[:, mc, :, 1::2]
                    pr = ppool.tile([128, 4, 32, 2], F32, tag="ps")
                    pi = ppool.tile([128, 4, 32, 2], F32, tag="ps")
                    cmm(pr[:], [getw(('R1_cos', L, c)),
                                getw(('R1_sinX', L, c))], [xr, xi])
                    cmm(pi[:], [getw(('R1_cos', L, c)),
                                getw(('R1_negsinX', L, c))], [xi, xr])
                    ps1[c] = (pr, pi)

                # ===== pi56 copies -> B1 (cos-part fused via stt) =====
                B1_r = spool.tile([128, 1024], F32, tag="B1r")
                B1_i = spool.tile([128, 1024], F32, tag="B1i")
                B1v_r = B1_r[:].rearrange("p (m h q) -> p m h q", m=8, h=32, q=4)
                B1v_i = B1_i[:].rearrange("p (m h q) -> p m h q", m=8, h=32, q=4)
                for comp, B0v, B1v in ((0, B0v_r, B1v_r), (1, B0v_i, B1v_i)):
                    copy(B1v[:, :, :, 0], B0v[:, :, :, 0])
                    copy(B1v[:, :, :, 3], B0v[:, :, :, 2])
                    for c in range(2):
                        mc = slice(4 * c, 4 * (c + 1))
                        p = ps1[c][comp]
                        copy(B1v[:, mc, :, 1], p[:, :, :, 0])
                        copy(B1v[:, mc, :, 2], p[:, :, :, 1])

                # ===== round 2: M2(b6) column groups =====
                B2_r = spool.tile([128, 1024], F32, tag="B2r")
                B2_i = spool.tile([128, 1024], F32, tag="B2i")
                B2v_r = B2_r[:].rearrange("p (m h q) -> p m h q", m=8, h=32, q=4)
                B2v_i = B2_i[:].rearrange("p (m h q) -> p m h q", m=8, h=32, q=4)
                absorb(B1_i[:, 896:])
                for c in range(2):
                    mc = slice(4 * c, 4 * (c + 1))
                    for b6 in (0, 1):
                        qs = slice(b6, 4, 2)
                        xr = B1v_r[:, mc, :, qs]
                        xi = B1v_i[:, mc, :, qs]
                        pr = ppool.tile([128, 4, 32, 2], F32, tag="ps")
                        pi = ppool.tile([128, 4, 32, 2], F32, tag="ps")
                        cmm(pr[:], [getw(('M2_rT', L, c, b6)),
                                    getw(('M2_negiT', L, c, b6))], [xr, xi])
                        cmm(pi[:], [getw(('M2_iT', L, c, b6)),
                                    getw(('M2_rT', L, c, b6))], [xr, xi])
                        copy(B2v_r[:, mc, :, qs], pr[:])
                        copy(B2v_i[:, mc, :, qs], pi[:])

                if L < NL - 1:
                    absorb(B2_i[:, 896:])
                    stA_r = spool.tile([128, 1024], F32, tag="stAr")
                    stA_i = spool.tile([128, 1024], F32, tag="stAi")
                    for m in range(8):
                        cs = slice(128 * m, 128 * (m + 1))
                        for src, dst in ((B2_r, stA_r), (B2_i, stA_i)):
                            pt = tpool.tile([128, 128], F32, tag="pt")
                            nc.tensor.transpose(pt[:], src[:, cs], ident[:])
                            copy(dst[:, cs], pt[:])

            # ===== final: circuit 1 (Z) in layout B =====
            p2z_t = opool.tile([128, 512], F32, tag="p2z")
            tmpz = spool.tile([128, 512], F32, tag="tmpz")
            sqz = spool.tile([128, 512], F32, tag="sqz")
            nc.scalar.activation(tmpz[:], B2_r[:, 512:], SQ)
            nc.scalar.activation(sqz[:], B2_i[:, 512:], SQ)
            nc.vector.tensor_add(p2z_t[:], sqz[:], tmpz[:])
            dma(nc.gpsimd, out=p2z_ext[:], in_=p2z_t[:])

            # ===== final: circuit 0 (X): back to A, apply H_hi =====
            absorb(B2_i[:, 896:])
            fA_r = spool.tile([128, 512], F32, tag="fAr")
            fA_i = spool.tile([128, 512], F32, tag="fAi")
            for m in range(4):
                cs = slice(128 * m, 128 * (m + 1))
                for src, dst in ((B2_r, fA_r), (B2_i, fA_i)):
                    pt = tpool.tile([128, 128], F32, tag="pt")
                    nc.tensor.transpose(pt[:], src[:, cs], ident[:])
                    copy(dst[:, cs], pt[:])
            absorb(fA_i[:, 384:])
            ph_r = ppool.tile([128, 512], F32, tag="ps")
            ph_i = ppool.tile([128, 512], F32, tag="ps")
            cmm(ph_r[:], [getw(('Hhi',))], [fA_r[:]])
            cmm(ph_i[:], [getw(('Hhi',))], [fA_i[:]])
            p2x_t = opool.tile([128, 512], F32, tag="p2x")
            tmpx = spool.tile([128, 512], F32, tag="tmpx")
            sqx = spool.tile([128, 512], F32, tag="sqx")
            nc.scalar.activation(tmpx[:], ph_r[:], SQ)
            last_act = nc.scalar.activation(sqx[:], ph_i[:], SQ)
            last_dve = nc.vector.tensor_add(p2x_t[:], sqx[:], tmpx[:])
            dma(nc.gpsimd, out=p2x_ext[:], in_=p2x_t[:])
            last_pe = nc.tensor.ldweights(jw[:])

            # The framework tail drain waits on every proc the sync engine
            # has not observed; each nop below absorbs one proc tick so the
            # drain itself stays under the 1-wait encoding limit.
            finale = [last_act, last_dve, last_pe] + dma_insts[-12:]
            for depi in finale:
                n = nc.sync.nop()
                add_dep_helper(n.ins, depi.ins, reason="tail tick absorb")

    return nc


def _get_program(nw, widx):
    key = ('prog', nw)
    if key not in _CACHE:
        _build_program.widx = widx
        _CACHE[key] = _build_program(nw)
    return _CACHE[key]


# ---------------- host <-> device glue ----------------
def _signs():
    j = np.arange(4096)
    S = np.empty((NQ, 4096), dtype=np.float32)
    for w in range(NQ):
        S[w] = 1.0 - 2.0 * ((j >> (NQ - 1 - w)) & 1)
    return S.reshape(NQ, 128, 32)


def _get_runner(nc):
    # Build the sharded PJRT callable once; run_bass_via_pjrt re-jits a new
    # closure per call (~1s of retrace overhead), so repeat kernel() calls
    # would pay that every time.
    if 'runner' in _CACHE:
        return _CACHE['runner']
    import jax
    import numpy as jnp_np
    from jax.sharding import Mesh, PartitionSpec
    from jax.experimental.shard_map import shard_map
    from concourse import bass2jax, mybir
    bass2jax.install_neuronx_cc_hook()
    _p = bass2jax._bass_exec_p

    pname = nc.partition_id_tensor.name if nc.partition_id_tensor else None
    in_names, out_names, out_avals, zero_outs = [], [], [], []
    for alloc in nc.m.functions[0].allocations:
        if not isinstance(alloc, mybir.MemoryLocationSet):
            continue
        name = alloc.memorylocations[0].name
        if alloc.kind == "ExternalInput":
            if name != pname:
                in_names.append(name)
        elif alloc.kind == "ExternalOutput":
            shape = tuple(alloc.tensor_shape)
            dtype = mybir.dt.np(alloc.dtype)
            out_names.append(name)
            out_avals.append(jax.core.ShapedArray(shape, dtype))
            zero_outs.append(np.zeros(shape, dtype))
    n_params = len(in_names)
    n_outs = len(out_avals)
    all_names = in_names + out_names
    if pname is not None:
        all_names = all_names + [pname]
    donate = tuple(range(n_params, n_params + n_outs))

    def _body(*args):
        operands = list(args)
        if pname is not None:
            operands.append(bass2jax.partition_id_tensor())
        outs = _p.bind(
            *operands, out_avals=tuple(out_avals), in_names=tuple(all_names),
            out_names=tuple(out_names), lowering_input_output_aliases=(),
            sim_require_finite=True, sim_require_nnan=True, nc=nc)
        return tuple(outs)

    devices = jax.devices()[:NCORES]
    mesh = Mesh(np.asarray(devices), ("core",))
    in_specs = (PartitionSpec("core"),) * (n_params + n_outs)
    out_specs = (PartitionSpec("core"),) * n_outs
    sharded = jax.jit(
        shard_map(_body, mesh=mesh, in_specs=in_specs, out_specs=out_specs,
                  check_rep=False),
        donate_argnums=donate, keep_unused=True)

    def run(in_maps):
        concat_in = [np.concatenate([m[n] for m in in_maps], axis=0)
                     for n in in_names]
        zo = [np.concatenate([z] * NCORES, axis=0) for z in zero_outs]
        outs = sharded(*concat_in, *zo)
        res = []
        for c in range(NCORES):
            d = {}
            for i, n in enumerate(out_names):
                arr = np.asarray(outs[i])
                per = arr.shape[0] // NCORES
                d[n] = arr[c * per:(c + 1) * per]
            res.append(d)
        return res

    _CACHE['runner'] = run
    return run


def kernel(x, rotations, cx_strengths, t_gates, _run_kwargs=None):

    x = np.asarray(x, dtype=np.float32)
    wts, widx = _weight_pack(np.asarray(rotations, dtype=np.float64),
                             np.asarray(cx_strengths, dtype=np.float64),
                             np.asarray(t_gates, dtype=np.float64))
    nw = wts.shape[0]
    nc = _get_program(nw, widx)

    # initial states, layout A per core: st0[h, s*32+l], s = c*16+n
    psi0 = _initial_state(x.astype(np.float64)).astype(np.float32)  # [128,4096]
    in_maps = []
    for k in range(NCORES):
        blk = psi0[k * BPC:(k + 1) * BPC].reshape(BPC, 128, 32)  # [n, h, l]
        st0 = np.empty((128, 1024), dtype=np.float32)
        v = st0.reshape(128, 32, 32)                              # [h, s, l]
        v[:, :BPC, :] = blk.transpose(1, 0, 2)
        v[:, BPC:, :] = blk.transpose(1, 0, 2)                    # same for c=1
        in_maps.append({"st0": st0, "wts": wts})

    results = _get_runner(nc)(in_maps)

    S = _signs()  # [w, h, l]
    out = np.empty((NCORES * BPC, 2 * NQ), dtype=np.float32)
    for k in range(NCORES):
        p2x = results[k]["p2x"]          # [h, n*32+l]
        p2z = results[k]["p2z"]          # [s4*32+l, m'*128+h]
        ex = np.einsum('hnl,whl->nw', p2x.reshape(128, BPC, 32), S,
                       optimize=True)
        p2zr = p2z.reshape(4, 32, 4, 128)          # [s4, l, m', h]
        ezn = np.einsum('slmh,whl->msw', p2zr, S, optimize=True)
        ez = ezn.reshape(BPC, NQ)                  # n = m'*4 + s4
        rows = slice(k * BPC, (k + 1) * BPC)
        out[rows, 0::2] = ex
        out[rows, 1::2] = ez
    return out

